# revision 7
# baseline (speedup 1.0000x reference)
"""Slot-attention kernel v3 for Trainium2, SPMD over 8 NeuronCores.

Reference computation (per batch element b):
  query[b,n,:] = q[n,b,:] @ qw[n]          (n = 32 query slots)
  keyp [b,m,:] = k[m,b,:] @ kw[m]          (m = 32 key slots)
  value[b,m,:] = k[m,b,:] @ vw[m]
  logits[b,n,m] = query[b,n,:]·keyp[b,m,:] / 16
  attn = softmax_m(logits)
  out[n,b,:] = sum_m attn[b,n,m] * value[b,m,:]

Sharding: data-parallel over batch (4096 -> 512 per core), weights replicated.

v3 design notes:
  - all-bf16 compute (fp8 V was 4.5e-2 rel err: value-quantization noise does
    not average out through the attention-weighted sum)
  - phase A matmuls column-split 4 ways (tile_position col strips): a strip's
    LDWEIGHTS overlaps the other strips' in-flight matmuls, hiding most of
    the serialized weight-load tax (ldw-opt is unavailable in this toolchain:
    bass emits standalone InstLdweights which walrus's ldw-opt rejects)
  - deep input prefetch (2-slot DMA groups, 3 buffers) to keep the PE gapless
    so the HAM clock gate stays at K=8/8 (v2 ran essentially all-cold because
    2-4us input stalls re-throttled the PE every ~10us)
  - V -> V32Q partition shuffle via HWDGE (nc.scalar.dma_start), overlapped
    under phase A; was a 120us GpSimd SWDGE dead zone in the baseline
  - exp/reduce/transpose batched to [128, 512] per 16-group quad
  - psum evacuations batched to FD-512, alternating Vector/Scalar engines
  - bf16 output, upcast on host
"""

import numpy as np
import ml_dtypes

import concourse.bass as bass
from concourse import bacc
import concourse.mybir as mybir
import concourse.tile as tile
from concourse.bass_utils import run_bass_kernel_spmd

BF16 = mybir.dt.bfloat16
F32 = mybir.dt.float32

NQ = 32          # query slots
NK = 32          # key slots
D = 256          # input dim (contraction of projections)
A = 256          # attn dim (contraction of logits)
O = 256          # out dim
BS = 4096
N_CORES = 8
BS_CORE = BS // N_CORES   # 512
SPLIT = 4        # column-split factor for phase-A matmuls (1, 2 or 4)


def build_kernel(bs_core=BS_CORE, n_halves=2, split=SPLIT):
    nc = bacc.Bacc()

    b_h = bs_core // n_halves          # batch per half (256)
    n_groups = b_h // 4                # 4-batch groups per half (64)
    n_quads = n_groups // 16           # exp/transpose batches (4)
    jstride = n_groups                 # batch stride between j-blocks (64)
    SG = 2                             # slots per input DMA group
    sw = 128 // split                  # strip width

    qT = nc.declare_dram_parameter("qT", [NQ, D, bs_core], BF16, isOutput=False)
    kT = nc.declare_dram_parameter("kT", [NK, D, bs_core], BF16, isOutput=False)
    wqk = nc.declare_dram_parameter("wqk", [NQ, D, 2, A], BF16, isOutput=False)
    wv = nc.declare_dram_parameter("wv", [NK, D, O], BF16, isOutput=False)
    out = nc.declare_dram_parameter("out", [NQ, bs_core, O], BF16, isOutput=True)

    # [slot, d, b] -> partition = d%128, chunk c = d//128
    qT_g = qT.rearrange("(sg s) (c p) b -> sg p (s c) b", p=128, s=SG)
    kT_g = kT.rearrange("(sg s) (c p) b -> sg p (s c) b", p=128, s=SG)
    wqk_g = wqk.rearrange("(sg s) (c p) w a -> sg p (s c) (w a)", p=128, s=SG)
    wv_g = wv.rearrange("(sg s) (c p) o -> sg p (s c) o", p=128, s=SG)

    with tile.TileContext(nc) as tc:
        with (
            tc.tile_pool(name="xin", bufs=3) as xin,
            tc.tile_pool(name="win", bufs=3) as win,
            tc.tile_pool(name="big", bufs=1) as big,
            tc.tile_pool(name="v32", bufs=2) as v32p,
            tc.tile_pool(name="vnp", bufs=4) as vnp,
            tc.tile_pool(name="eqp", bufs=3) as eqp,
            tc.tile_pool(name="tep", bufs=6) as tep,
            tc.tile_pool(name="smp", bufs=3) as smp,
            tc.tile_pool(name="outp", bufs=3) as outp,
            tc.tile_pool(name="ps", bufs=8, space="PSUM") as psp,
        ):
            rs = big.tile([128, n_halves, n_quads, 16], F32, tag="rs")

            for half in range(n_halves):
                b0 = half * b_h
                QTs = big.tile([128, 2, NQ, b_h], BF16, tag="QTs")
                KTs = big.tile([128, 2, NK, b_h], BF16, tag="KTs")
                V32Q = v32p.tile([128, n_groups, O], BF16, tag="V32Q")

                # ---- Phase A: projections ----
                for sg in range(NQ // SG):
                    qts = xin.tile([128, SG, 2, b_h], BF16, tag="qts")
                    nc.sync.dma_start(out=qts, in_=qT_g[sg, :, :, b0:b0 + b_h])
                    kts = xin.tile([128, SG, 2, b_h], BF16, tag="kts")
                    nc.sync.dma_start(out=kts, in_=kT_g[sg, :, :, b0:b0 + b_h])
                    wsg = win.tile([128, SG, 2, 2 * A], BF16, tag="wsg")
                    nc.sync.dma_start(out=wsg, in_=wqk_g[sg])
                    wvs = win.tile([128, SG, 2, O], BF16, tag="wvs")
                    nc.sync.dma_start(out=wvs, in_=wv_g[sg])

                    for si in range(SG):
                        s = sg * SG + si
                        # Q / K projections: weight-stationary, col-split
                        for pi in range(2):
                            xs = qts if pi == 0 else kts
                            dst = QTs if pi == 0 else KTs
                            ps = psp.tile([128, 2, b_h], F32, tag="bank")
                            for t in range(2):      # a-tile
                                for c in range(2):  # contraction chunk
                                    for u in range(split):
                                        nc.tensor.matmul(
                                            ps[sw * u:sw * (u + 1), t, :],
                                            lhsT=wsg[:, si, c,
                                                     pi * A + t * 128 + sw * u:
                                                     pi * A + t * 128 + sw * (u + 1)],
                                            rhs=xs[:, si, c, :],
                                            start=(c == 0),
                                            stop=(c == 1),
                                            tile_position=(0, sw * u),
                                            skip_group_check=True,
                                        )
                            # evac psum -> sbuf (1/16 temperature folded into Q)
                            if pi == 0:
                                if s % 2 == 0:
                                    nc.scalar.mul(dst[:, :, s, :], ps, 1.0 / 16.0)
                                else:
                                    nc.vector.tensor_scalar_mul(
                                        out=dst[:, :, s, :], in0=ps,
                                        scalar1=1.0 / 16.0)
                            else:
                                if s % 2 == 1:
                                    nc.scalar.copy(out=dst[:, :, s, :], in_=ps)
                                else:
                                    nc.vector.tensor_copy(out=dst[:, :, s, :],
                                                          in_=ps)

                        # V projection: x-stationary, col-split over b
                        vps = psp.tile([128, 2, O], F32, tag="bank")
                        for bc in range(2):
                            for c in range(2):
                                for u in range(split):
                                    nc.tensor.matmul(
                                        vps[sw * u:sw * (u + 1), bc, :],
                                        lhsT=kts[:, si, c,
                                                 bc * 128 + sw * u:
                                                 bc * 128 + sw * (u + 1)],
                                        rhs=wvs[:, si, c, :],
                                        start=(c == 0),
                                        stop=(c == 1),
                                        tile_position=(0, sw * u),
                                        skip_group_check=True,
                                    )
                        vn = vnp.tile([128, 2, O], BF16, tag="vn")
                        if s % 2 == 0:
                            nc.scalar.copy(out=vn, in_=vps)
                        else:
                            nc.vector.tensor_copy(out=vn, in_=vps)
                        # shuffle V[b-local = bc*128 + p][m=s, o] ->
                        #   V32Q[32*j + s, g, o], j = b-local//64, g = b-local%64
                        for bc in range(2):
                            for jh in range(2):
                                j = 2 * bc + jh
                                nc.scalar.dma_start(
                                    out=V32Q[32 * j + s:32 * j + s + 1, :, :],
                                    in_=vn[64 * jh:64 * (jh + 1), bc, :],
                                )

                # ---- Phase B + C, software-pipelined: C(qd-1) is
                # emitted after B(qd) so the PE has logit matmuls to chew on
                # while exp/reduce/transpose of quad qd run on Scalar/Vector.
                TEs = [None] * n_quads

                def phase_c(qd, TEq):
                    for g4 in range(8):
                        av = psp.tile([128, 2, O], F32, tag="bank")
                        for gi in range(2):
                            g = qd * 16 + g4 * 2 + gi
                            for j in range(4):
                                nc.tensor.matmul(
                                    av[32 * j:32 * (j + 1), gi, :],
                                    lhsT=TEq[32 * j:32 * (j + 1),
                                             g4 * 2 + gi, :],
                                    rhs=V32Q[32 * j:32 * (j + 1), g, :],
                                    start=True, stop=True,
                                    tile_position=(32 * j, 32 * j),
                                    skip_group_check=True,
                                )
                        OUTo = outp.tile([128, 2, O], BF16, tag="OUTo")
                        for gi in range(2):
                            q16 = g4 * 2 + gi
                            if gi == 0:
                                nc.scalar.mul(OUTo[:, gi, :], av[:, gi, :],
                                              rs[:, half, qd, q16:q16 + 1])
                            else:
                                nc.vector.tensor_scalar_mul(
                                    out=OUTo[:, gi, :], in0=av[:, gi, :],
                                    scalar1=rs[:, half, qd, q16:q16 + 1])
                        g0 = qd * 16 + g4 * 2
                        for j in range(4):
                            nc.sync.dma_start(
                                out=out[:, b0 + jstride * j + g0:
                                        b0 + jstride * j + g0 + 2, :],
                                in_=OUTo[32 * j:32 * (j + 1), :, :],
                            )

                for qd in range(n_quads):
                    lg = psp.tile([128, 16, NK], F32, tag="bank")
                    for gi in range(16):
                        g = qd * 16 + gi
                        for c in range(2):
                            for j in range(4):
                                b = g + jstride * j
                                nc.tensor.matmul(
                                    lg[32 * j:32 * (j + 1), gi, :],
                                    lhsT=QTs[:, c, :, b],
                                    rhs=KTs[:, c, :, b],
                                    start=(c == 0),
                                    stop=(c == 1),
                                    tile_position=(0, 32 * j),
                                    skip_group_check=True,
                                )
                    Eq = eqp.tile([128, 16, NK], BF16, tag="Eq")
                    nc.scalar.activation(
                        out=Eq.rearrange("p a b -> p (a b)"),
                        in_=lg.rearrange("p a b -> p (a b)"),
                        func=mybir.ActivationFunctionType.Exp,
                    )
                    sm = smp.tile([128, 16], F32, tag="sm")
                    nc.vector.reduce_sum(out=sm, in_=Eq,
                                         axis=mybir.AxisListType.X)
                    nc.vector.reciprocal(out=rs[:, half, qd, :], in_=sm)
                    TEq = tep.tile([128, 16, NQ], BF16, tag="TEq")
                    nc.vector.transpose(out=TEq, in_=Eq)
                    TEs[qd] = TEq
                    if qd > 0:
                        phase_c(qd - 1, TEs[qd - 1])
                phase_c(n_quads - 1, TEs[n_quads - 1])
    return nc


def _prep_inputs(q, k, query_weight, key_weight, value_weight, bs_core):
    bf = ml_dtypes.bfloat16
    wqk = np.ascontiguousarray(
        np.stack((query_weight, key_weight), axis=2)).astype(bf)
    wvb = np.ascontiguousarray(value_weight).astype(bf)
    in_maps = []
    for i in range(N_CORES):
        sl = slice(i * bs_core, (i + 1) * bs_core)
        qTb = np.ascontiguousarray(q[:, sl, :].transpose(0, 2, 1)).astype(bf)
        kTb = np.ascontiguousarray(k[:, sl, :].transpose(0, 2, 1)).astype(bf)
        in_maps.append({"qT": qTb, "kT": kTb, "wqk": wqk, "wv": wvb})
    return in_maps


_NC_CACHE = {}


def _get_nc(bs_core):
    if bs_core not in _NC_CACHE:
        nc = build_kernel(bs_core)
        nc.finalize()
        _NC_CACHE[bs_core] = nc
    return _NC_CACHE[bs_core]


def kernel(q, k, query_weight, key_weight, value_weight, _trace=False):
    nc = _get_nc(BS_CORE)
    in_maps = _prep_inputs(q, k, query_weight, key_weight, value_weight,
                           BS_CORE)
    res = run_bass_kernel_spmd(nc, in_maps, core_ids=list(range(N_CORES)),
                               trace=_trace)
    outs = [res.results[i]["out"] for i in range(N_CORES)]
    full = np.concatenate(outs, axis=1).astype(np.float32)
    if _trace:
        return full, res
    return full


# revision 8
# speedup vs baseline: 1.3727x; 1.3727x over previous
"""Slot-attention kernel v3 for Trainium2, SPMD over 8 NeuronCores.

Reference computation (per batch element b):
  query[b,n,:] = q[n,b,:] @ qw[n]          (n = 32 query slots)
  keyp [b,m,:] = k[m,b,:] @ kw[m]          (m = 32 key slots)
  value[b,m,:] = k[m,b,:] @ vw[m]
  logits[b,n,m] = query[b,n,:]·keyp[b,m,:] / 16
  attn = softmax_m(logits)
  out[n,b,:] = sum_m attn[b,n,m] * value[b,m,:]

Sharding: data-parallel over batch (4096 -> 512 per core), weights replicated.

v3 design notes:
  - all-bf16 compute (fp8 V was 4.5e-2 rel err: value-quantization noise does
    not average out through the attention-weighted sum)
  - phase A matmuls column-split 4 ways (tile_position col strips): a strip's
    LDWEIGHTS overlaps the other strips' in-flight matmuls, hiding most of
    the serialized weight-load tax (ldw-opt is unavailable in this toolchain:
    bass emits standalone InstLdweights which walrus's ldw-opt rejects)
  - deep input prefetch (2-slot DMA groups, 3 buffers) to keep the PE gapless
    so the HAM clock gate stays at K=8/8 (v2 ran essentially all-cold because
    2-4us input stalls re-throttled the PE every ~10us)
  - V -> V32Q partition shuffle via HWDGE (nc.scalar.dma_start), overlapped
    under phase A; was a 120us GpSimd SWDGE dead zone in the baseline
  - exp/reduce/transpose batched to [128, 512] per 16-group quad
  - psum evacuations batched to FD-512, alternating Vector/Scalar engines
  - bf16 output, upcast on host
"""

import numpy as np
import ml_dtypes

import concourse.bass as bass
from concourse import bacc
import concourse.mybir as mybir
import concourse.tile as tile
from concourse.bass_utils import run_bass_kernel_spmd

BF16 = mybir.dt.bfloat16
F32 = mybir.dt.float32

NQ = 32          # query slots
NK = 32          # key slots
D = 256          # input dim (contraction of projections)
A = 256          # attn dim (contraction of logits)
O = 256          # out dim
BS = 4096
N_CORES = 8
BS_CORE = BS // N_CORES   # 512
SPLIT = 1        # column-split factor for phase-A matmuls (1, 2 or 4)


def build_kernel(bs_core=BS_CORE, n_halves=2, split=SPLIT):
    nc = bacc.Bacc()

    b_h = bs_core // n_halves          # batch per half (256)
    n_groups = b_h // 4                # 4-batch groups per half (64)
    n_quads = n_groups // 16           # exp/transpose batches (4)
    jstride = n_groups                 # batch stride between j-blocks (64)
    SG = 2                             # slots per input DMA group
    sw = 128 // split                  # strip width

    qT = nc.declare_dram_parameter("qT", [NQ, D, bs_core], BF16, isOutput=False)
    kT = nc.declare_dram_parameter("kT", [NK, D, bs_core], BF16, isOutput=False)
    wqk = nc.declare_dram_parameter("wqk", [NQ, D, 2, A], BF16, isOutput=False)
    wv = nc.declare_dram_parameter("wv", [NK, D, O], BF16, isOutput=False)
    out = nc.declare_dram_parameter("out", [NQ, bs_core, O], BF16, isOutput=True)

    # [slot, d, b] -> partition = d%128, chunk c = d//128
    qT_g = qT.rearrange("(sg s) (c p) b -> sg p (s c) b", p=128, s=SG)
    kT_g = kT.rearrange("(sg s) (c p) b -> sg p (s c) b", p=128, s=SG)
    wqk_g = wqk.rearrange("(sg s) (c p) w a -> sg p (s c) (w a)", p=128, s=SG)
    wv_g = wv.rearrange("(sg s) (c p) o -> sg p (s c) o", p=128, s=SG)

    with tile.TileContext(nc) as tc:
        with (
            tc.tile_pool(name="xin", bufs=4) as xin,
            tc.tile_pool(name="win", bufs=3) as win,
            tc.tile_pool(name="big", bufs=1) as big,
            tc.tile_pool(name="v32", bufs=2) as v32p,
            tc.tile_pool(name="vnp", bufs=4) as vnp,
            tc.tile_pool(name="eqp", bufs=3) as eqp,
            tc.tile_pool(name="tep", bufs=6) as tep,
            tc.tile_pool(name="smp", bufs=3) as smp,
            tc.tile_pool(name="outp", bufs=3) as outp,
            tc.tile_pool(name="ps", bufs=8, space="PSUM") as psp,
        ):
            rs = big.tile([128, n_halves, n_quads, 16], F32, tag="rs")

            for half in range(n_halves):
                b0 = half * b_h
                QTs = big.tile([128, 2, NQ, b_h], BF16, tag="QTs")
                KTs = big.tile([128, 2, NK, b_h], BF16, tag="KTs")
                V32Q = v32p.tile([128, n_groups, O], BF16, tag="V32Q")

                # ---- Phase A: projections ----
                for sg in range(NQ // SG):
                    qts = xin.tile([128, SG, 2, b_h], BF16, tag="qts")
                    nc.sync.dma_start(out=qts, in_=qT_g[sg, :, :, b0:b0 + b_h])
                    kts = xin.tile([128, SG, 2, b_h], BF16, tag="kts")
                    nc.sync.dma_start(out=kts, in_=kT_g[sg, :, :, b0:b0 + b_h])
                    wsg = win.tile([128, SG, 2, 2 * A], BF16, tag="wsg")
                    nc.sync.dma_start(out=wsg, in_=wqk_g[sg])
                    wvs = win.tile([128, SG, 2, O], BF16, tag="wvs")
                    nc.sync.dma_start(out=wvs, in_=wv_g[sg])

                    for si in range(SG):
                        s = sg * SG + si
                        # Q / K projections: weight-stationary, col-split
                        for pi in range(2):
                            xs = qts if pi == 0 else kts
                            dst = QTs if pi == 0 else KTs
                            ps = psp.tile([128, 2, b_h], F32, tag="bank")
                            for t in range(2):      # a-tile
                                for c in range(2):  # contraction chunk
                                    for u in range(split):
                                        nc.tensor.matmul(
                                            ps[sw * u:sw * (u + 1), t, :],
                                            lhsT=wsg[:, si, c,
                                                     pi * A + t * 128 + sw * u:
                                                     pi * A + t * 128 + sw * (u + 1)],
                                            rhs=xs[:, si, c, :],
                                            start=(c == 0),
                                            stop=(c == 1),
                                            tile_position=(0, sw * u),
                                            skip_group_check=True,
                                        )
                            # evac psum -> sbuf (1/16 temperature folded into Q)
                            if pi == 0:
                                if s % 2 == 0:
                                    nc.scalar.mul(dst[:, :, s, :], ps, 1.0 / 16.0)
                                else:
                                    nc.vector.tensor_scalar_mul(
                                        out=dst[:, :, s, :], in0=ps,
                                        scalar1=1.0 / 16.0)
                            else:
                                if s % 2 == 1:
                                    nc.scalar.copy(out=dst[:, :, s, :], in_=ps)
                                else:
                                    nc.vector.tensor_copy(out=dst[:, :, s, :],
                                                          in_=ps)

                        # V projection: x-stationary, col-split over b
                        vps = psp.tile([128, 2, O], F32, tag="bank")
                        for bc in range(2):
                            for c in range(2):
                                for u in range(split):
                                    nc.tensor.matmul(
                                        vps[sw * u:sw * (u + 1), bc, :],
                                        lhsT=kts[:, si, c,
                                                 bc * 128 + sw * u:
                                                 bc * 128 + sw * (u + 1)],
                                        rhs=wvs[:, si, c, :],
                                        start=(c == 0),
                                        stop=(c == 1),
                                        tile_position=(0, sw * u),
                                        skip_group_check=True,
                                    )
                        vn = vnp.tile([128, 2, O], BF16, tag="vn")
                        if s % 2 == 0:
                            nc.scalar.copy(out=vn, in_=vps)
                        else:
                            nc.vector.tensor_copy(out=vn, in_=vps)
                        # shuffle V[b-local = bc*128 + p][m=s, o] ->
                        #   V32Q[32*j + s, g, o], j = b-local//64, g = b-local%64
                        for bc in range(2):
                            for jh in range(2):
                                j = 2 * bc + jh
                                nc.scalar.dma_start(
                                    out=V32Q[32 * j + s:32 * j + s + 1, :, :],
                                    in_=vn[64 * jh:64 * (jh + 1), bc, :],
                                )

                # ---- Phase B + C, software-pipelined: C(qd-1) is
                # emitted after B(qd) so the PE has logit matmuls to chew on
                # while exp/reduce/transpose of quad qd run on Scalar/Vector.
                TEs = [None] * n_quads

                def phase_c(qd, TEq):
                    for g8 in range(2):
                        OUTo = outp.tile([128, 8, O], BF16, tag="OUTo")
                        for g4 in range(4):
                            av = psp.tile([128, 2, O], F32, tag="bank")
                            for gi in range(2):
                                g = qd * 16 + g8 * 8 + g4 * 2 + gi
                                for j in range(4):
                                    nc.tensor.matmul(
                                        av[32 * j:32 * (j + 1), gi, :],
                                        lhsT=TEq[32 * j:32 * (j + 1),
                                                 g8 * 8 + g4 * 2 + gi, :],
                                        rhs=V32Q[32 * j:32 * (j + 1), g, :],
                                        start=True, stop=True,
                                        tile_position=(32 * j, 32 * j),
                                        skip_group_check=True,
                                    )
                            for gi in range(2):
                                q16 = g8 * 8 + g4 * 2 + gi
                                if gi == 0:
                                    nc.scalar.mul(OUTo[:, g4 * 2 + gi, :],
                                                  av[:, gi, :],
                                                  rs[:, half, qd, q16:q16 + 1])
                                else:
                                    nc.vector.tensor_scalar_mul(
                                        out=OUTo[:, g4 * 2 + gi, :],
                                        in0=av[:, gi, :],
                                        scalar1=rs[:, half, qd, q16:q16 + 1])
                        g0 = qd * 16 + g8 * 8
                        for j in range(4):
                            nc.sync.dma_start(
                                out=out[:, b0 + jstride * j + g0:
                                        b0 + jstride * j + g0 + 8, :],
                                in_=OUTo[32 * j:32 * (j + 1), :, :],
                            )

                for qd in range(n_quads):
                    lg = psp.tile([128, 16, NK], F32, tag="bank")
                    for gi in range(16):
                        g = qd * 16 + gi
                        for c in range(2):
                            for j in range(4):
                                b = g + jstride * j
                                nc.tensor.matmul(
                                    lg[32 * j:32 * (j + 1), gi, :],
                                    lhsT=QTs[:, c, :, b],
                                    rhs=KTs[:, c, :, b],
                                    start=(c == 0),
                                    stop=(c == 1),
                                    tile_position=(0, 32 * j),
                                    skip_group_check=True,
                                )
                    Eq = eqp.tile([128, 16, NK], BF16, tag="Eq")
                    nc.scalar.activation(
                        out=Eq.rearrange("p a b -> p (a b)"),
                        in_=lg.rearrange("p a b -> p (a b)"),
                        func=mybir.ActivationFunctionType.Exp,
                    )
                    sm = smp.tile([128, 16], F32, tag="sm")
                    nc.vector.reduce_sum(out=sm, in_=Eq,
                                         axis=mybir.AxisListType.X)
                    nc.vector.reciprocal(out=rs[:, half, qd, :], in_=sm)
                    TEq = tep.tile([128, 16, NQ], BF16, tag="TEq")
                    nc.vector.transpose(out=TEq, in_=Eq)
                    TEs[qd] = TEq
                    if qd > 0:
                        phase_c(qd - 1, TEs[qd - 1])
                phase_c(n_quads - 1, TEs[n_quads - 1])
    return nc


def _prep_inputs(q, k, query_weight, key_weight, value_weight, bs_core):
    bf = ml_dtypes.bfloat16
    wqk = np.ascontiguousarray(
        np.stack((query_weight, key_weight), axis=2)).astype(bf)
    wvb = np.ascontiguousarray(value_weight).astype(bf)
    in_maps = []
    for i in range(N_CORES):
        sl = slice(i * bs_core, (i + 1) * bs_core)
        qTb = np.ascontiguousarray(q[:, sl, :].transpose(0, 2, 1)).astype(bf)
        kTb = np.ascontiguousarray(k[:, sl, :].transpose(0, 2, 1)).astype(bf)
        in_maps.append({"qT": qTb, "kT": kTb, "wqk": wqk, "wv": wvb})
    return in_maps


_NC_CACHE = {}


def _get_nc(bs_core):
    if bs_core not in _NC_CACHE:
        nc = build_kernel(bs_core)
        nc.finalize()
        _NC_CACHE[bs_core] = nc
    return _NC_CACHE[bs_core]


def kernel(q, k, query_weight, key_weight, value_weight, _trace=False):
    nc = _get_nc(BS_CORE)
    in_maps = _prep_inputs(q, k, query_weight, key_weight, value_weight,
                           BS_CORE)
    res = run_bass_kernel_spmd(nc, in_maps, core_ids=list(range(N_CORES)),
                               trace=_trace)
    outs = [res.results[i]["out"] for i in range(N_CORES)]
    full = np.concatenate(outs, axis=1).astype(np.float32)
    if _trace:
        return full, res
    return full
